# revision 7
# baseline (speedup 1.0000x reference)
"""Trainium2 Bass kernel for nn_DynamicConv (dense_cnn).

Math: the 12 scaled conv kernels (lengths 3..11, 1->4 channels) collapse by
linearity into one 11-tap FIR per channel; only the last 8 output positions
use masked (shorter) kernel sets, and the last 2 positions of the padded
length are dropped.  The attention MLP that produces the 12 softmax weights
reads only x[0] and is computed on host (0.03% of FLOPs); its result is baked
into the FIR taps passed to the device.

Device strategy (8 cores, batch-parallel, 4 batches/core):
  - x[b] viewed as 128 chunks x F (F = L/128).  PE transposes 128-column
    slices into "run-major" tiles XT[p, (b,a)] = x[b, a*F + c0 + p].
  - Conv = banded-Toeplitz matmul: psum[q, (b,a)] = A_c^T @ XT (+ B_c^T @
    XT_next rows 0..9 for taps crossing the 128 boundary), PSUM-accumulated.
  - bn_stats on every conv psum tile -> per-partition mean/M2; aggregated,
    reduced over partitions by a ones-matmul, AllReduce'd across the 8 cores
    (tiny [1,8] collective), then scale/shift = f(gamma, beta, mean, var).
  - Phase 2: PE back-transposes grouped 8-per-2-bank-PSUM so the fused
    BN-affine+ReLU runs as few wide ScalarE ops while copying PSUM->SBUF
    staging; contiguous 1MB output DMAs alternate across both HWDGE rings.
  - Default matmuls are exact fp32 (4 cyc/row on the PE); KERNEL_F32R=1
    switches the conv to the TF32 path (1 cyc/row, ~2e-4 max rel err).
"""

import os
import numpy as np

import concourse.bass as bass
import concourse.bacc as bacc
import concourse.tile as tile
from concourse import mybir
from concourse import bass_utils

KS = [3, 3, 3, 5, 5, 7, 7, 7, 9, 9, 11, 11]
B, L, CH = 32, 262144, 4
N_CORES = 8
EPS = 1e-5
MAXK = 11

F32 = mybir.dt.float32
F32R = mybir.dt.float32r


# ---------------------------------------------------------------- host math
def _attention_weights(x0, w1, b1, w2, b2):
    """softmax weights s[12] from batch element 0 (reference uses y[0,0])."""
    n = x0.shape[0]
    pooled = x0.reshape(256, n // 256).mean(axis=1)
    h = np.maximum(w1.astype(np.float32) @ pooled + b1, 0.0).astype(np.float32)
    z = (w2.astype(np.float32) @ h + b2).astype(np.float32)
    z = z - z.max()
    e = np.exp(z)
    return (e / e.sum()).astype(np.float32)


def _fir_taps(s, kernels):
    """K_eff[c, j] and the 8 masked tail variants K_tail[d][c, j]."""
    keff = np.zeros((CH, MAXK), np.float32)
    for i, k in enumerate(kernels):
        keff[:, : KS[i]] += s[i] * k[:, 0, :]
    ktail = np.zeros((8, CH, MAXK), np.float32)
    for d in range(8):
        for i, k in enumerate(kernels):
            if KS[i] <= 10 - d:
                ktail[d, :, : KS[i]] += s[i] * k[:, 0, :]
    return keff, ktail


def _toeplitz_mats(keff, ktail):
    """lhsT matrices for the banded conv, boundary corner, and masked tail."""
    A = np.zeros((CH, 128, 128), np.float32)
    Bm = np.zeros((CH, 10, 128), np.float32)
    for c in range(CH):
        for p in range(128):
            for q in range(max(0, p - (MAXK - 1)), p + 1):
                A[c, p, q] = keff[c, p - q]
        for r in range(10):
            for q in range(118 + r, 128):
                j = 128 - q + r
                if j < MAXK:
                    Bm[c, r, q] = keff[c, j]
    # correction for the last 128-column of the last slice: psum there holds
    # A.T@u (B side saw zeros).  D = T - A fixes q in [118,126) to the masked
    # tail value and exactly cancels q in {126,127} (dropped positions).
    D = np.zeros((CH, 128, 128), np.float32)
    for c in range(CH):
        T = np.zeros((128, 128), np.float32)
        for q in range(118, 126):
            d = q - 118
            for j in range(MAXK):
                p = q + j
                if p < 128:
                    T[p, q] = ktail[d, c, j]
        D[c, :, 118:] = T[:, 118:] - A[c, :, 118:]
    return A, Bm, D


# ---------------------------------------------------------------- device IR
def _build(n_cores, b_loc, length, conv_f32r, no_collective=False, repeat=1):
    """repeat>1 wraps the whole kernel body in a tc.For_i hardware loop —
    a timing-only variant that executes the identical (idempotent) kernel
    `repeat` times back-to-back on device, so steady-state per-pass time can
    be measured above the ~70-100 ms axon dispatch round-trip noise."""
    F = length // 128            # chunk length per partition row
    NS = F // 128                # number of 128-column slices
    ML = length - 2              # valid output length
    NTOT = float(n_cores * b_loc * ML)
    n_per_part = 512 * NS        # bn_stats element count per partition/channel

    nc = bacc.Bacc("TRN2", target_bir_lowering=False, debug=False,
                   num_devices=1 if no_collective else n_cores)

    x_d = nc.dram_tensor("x", [b_loc, length], F32, kind="ExternalInput")
    a_d = nc.dram_tensor("amat", [CH, 128, 128], F32, kind="ExternalInput")
    b_d = nc.dram_tensor("bmat", [CH, 10, 128], F32, kind="ExternalInput")
    d_d = nc.dram_tensor("dmat", [CH, 128, 128], F32, kind="ExternalInput")
    gb_d = nc.dram_tensor("gb", [1, 8], F32, kind="ExternalInput")
    out_d = nc.dram_tensor("out", [b_loc, CH, ML], F32, kind="ExternalOutput")

    wdt = F32R if conv_f32r else F32

    from contextlib import nullcontext
    with tile.TileContext(nc) as tc:
        with (tc.For_i(0, repeat, 1) if repeat > 1 else nullcontext()), \
             tc.tile_pool(name="singles", bufs=1) as singles, \
             tc.tile_pool(name="xpool", bufs=1) as xpool, \
             tc.tile_pool(name="cpool", bufs=NS * CH) as cpool, \
             tc.tile_pool(name="stats", bufs=1) as stats, \
             tc.tile_pool(name="dram", bufs=1, space="DRAM") as dram:

            ident = nc.inline_tensor(np.eye(128, dtype=np.float32), name="ident")
            ident_sb = singles.tile([128, 128], F32, tag="ident")
            nc.sync.dma_start(out=ident_sb, in_=ident.ap())
            ones = nc.inline_tensor(np.ones((128, 1), np.float32), name="ones")
            ones_sb = singles.tile([128, 1], F32, tag="ones")
            nc.sync.dma_start(out=ones_sb, in_=ones.ap())

            # conv weight matrices (cast to f32r on device when enabled)
            a_sb, b_sb = [], []
            for c in range(CH):
                at = singles.tile([128, 128], F32, tag=f"a{c}")
                nc.sync.dma_start(out=at, in_=a_d.ap()[c])
                bt = singles.tile([10, 128], F32, tag=f"b{c}")
                nc.sync.dma_start(out=bt, in_=b_d.ap()[c])
                if conv_f32r:
                    ar = singles.tile([128, 128], F32R, tag=f"ar{c}")
                    nc.vector.tensor_copy(ar[:], at[:])
                    br = singles.tile([10, 128], F32R, tag=f"br{c}")
                    nc.vector.tensor_copy(br[:], bt[:])
                    a_sb.append(ar)
                    b_sb.append(br)
                else:
                    a_sb.append(at)
                    b_sb.append(bt)
            d_sb = []
            for c in range(CH):
                dt_ = singles.tile([128, 128], F32, tag=f"d{c}", name=f"d{c}")
                nc.sync.dma_start(out=dt_, in_=d_d.ap()[c])
                if conv_f32r:
                    dr = singles.tile([128, 128], F32R, tag=f"dr{c}",
                                      name=f"dr{c}")
                    nc.vector.tensor_copy(dr[:], dt_[:])
                    d_sb.append(dr)
                else:
                    d_sb.append(dt_)
            gb_sb = singles.tile([1, 8], F32, tag="gb")
            nc.sync.dma_start(out=gb_sb, in_=gb_d.ap())

            # x tiles [128, F+10] with inter-chunk halo.  Loads round-robin
            # over the three DMA-issue paths (2 HWDGE rings + SWDGE): a single
            # ring sustains only ~30 GB/s here, three in parallel ~125 GB/s.
            dma_engs = [nc.sync, nc.scalar, nc.gpsimd]
            x_tiles = []
            for b in range(b_loc):
                xt = xpool.tile([128, F + 10], F32, tag=f"X{b}")
                xv = x_d.ap()[b].rearrange("(a f) -> a f", f=F)
                nq = int(os.environ.get("KERNEL_XSPLIT", "3"))
                for qi in range(nq):
                    f0, f1 = qi * F // nq, (qi + 1) * F // nq
                    eng = dma_engs[(b * nq + qi) % 3]
                    eng.dma_start(out=xt[:, f0:f1], in_=xv[:, f0:f1])
                nc.vector.memset(xt[:, F:F + 10], 0.0)
                nc.sync.dma_start(
                    out=xt[0:127, F:F + 10],
                    in_=x_d.ap()[b, F:length].rearrange(
                        "(a f) -> a f", f=F)[:, 0:10])
                x_tiles.append(xt)

            # per-channel bn_stats collection
            bnst = [stats.tile([128, NS, 6], F32, tag=f"bnst{c}", name=f"bnst{c}")
                    for c in range(CH)]
            c_tiles = [[None] * NS for _ in range(CH)]

            with tc.tile_pool(name="xt", bufs=int(os.environ.get("KERNEL_XT", "4"))) as xtp, \
                 tc.tile_pool(name="ht", bufs=1) as htp, \
                 tc.tile_pool(name="psT", bufs=2, space="PSUM") as psT, \
                 tc.tile_pool(name="psC", bufs=5, space="PSUM") as psC, \
                 tc.tile_pool(name="psS", bufs=1, space="PSUM") as psS:

                def make_xt(si):
                    c0 = si * 128
                    pst = psT.tile([128, 512], F32, tag="pst")
                    for b in range(b_loc):
                        nc.tensor.transpose(
                            pst[:, b * 128:(b + 1) * 128],
                            x_tiles[b][:, c0:c0 + 128], ident_sb[:])
                    xt4 = xtp.tile([128, 512], wdt, tag="xt4")
                    nc.scalar.copy(out=xt4[:], in_=pst[:])
                    return xt4

                def make_ht():
                    psh = psS.tile([16, 512], F32, tag="small", name="psh")
                    for b in range(b_loc):
                        nc.tensor.transpose(
                            psh[0:10, b * 128:(b + 1) * 128],
                            x_tiles[b][:, F:F + 10], ident_sb[:])
                    ht4 = htp.tile([10, 512], wdt, tag="ht4")
                    nc.vector.tensor_copy(ht4[:], psh[0:10, :])
                    return ht4

                xt_cur = make_xt(0)
                for si in range(NS):
                    last = si == NS - 1
                    xt_nxt = make_ht() if last else make_xt(si + 1)
                    for c in range(CH):
                        pc = psC.tile([128, 512], F32, tag="pc")
                        nc.tensor.matmul(pc[:], a_sb[c][:], xt_cur[:],
                                         start=True, stop=False)
                        nc.tensor.matmul(pc[:], b_sb[c][:], xt_nxt[0:10, :],
                                         start=False, stop=True)
                        if last:
                            pc3 = pc.rearrange("q (b a) -> q b a", b=b_loc)
                            u4 = xt_cur.rearrange(
                                "p (b a) -> p b a", b=b_loc)[:, :, 127]
                            ps_d = psS.tile([128, b_loc], F32, tag="small",
                                            name="ps_d")
                            nc.tensor.matmul(ps_d[:], d_sb[c][:], u4,
                                             start=True, stop=True)
                            sd_sb = stats.tile([128, b_loc], F32,
                                               tag="sdfix", bufs=2,
                                               name="sdfix")
                            nc.vector.tensor_copy(sd_sb[:], ps_d[:])
                            nc.vector.tensor_add(
                                pc3[:, :, 127], pc3[:, :, 127], sd_sb[:])
                        nc.vector.bn_stats(out=bnst[c][:, si, :], in_=pc[:])
                        ct = cpool.tile([128, 512], F32, tag="ct")
                        n_dve = int(os.environ.get("KERNEL_CDVE", "1"))
                        if c >= n_dve:
                            nc.scalar.copy(out=ct[:], in_=pc[:])
                        else:
                            nc.vector.tensor_copy(ct[:], pc[:])
                        c_tiles[c][si] = ct
                    xt_cur = xt_nxt

                # ---- stats finalize + collective
                stats_loc = stats.tile([128, 8], F32, tag="stats_loc")
                for c in range(CH):
                    mv = stats.tile([128, 2], F32, tag=f"mv{c}")
                    nc.vector.bn_aggr(out=mv[:], in_=bnst[c][:])
                    msq = stats.tile([128, 1], F32, tag=f"msq{c}")
                    nc.vector.tensor_mul(msq[:], mv[:, 0:1], mv[:, 0:1])
                    e2 = stats.tile([128, 1], F32, tag=f"e2{c}")
                    nc.vector.tensor_add(e2[:], mv[:, 1:2], msq[:])
                    nc.scalar.mul(out=stats_loc[:, c:c + 1], in_=mv[:, 0:1],
                                  mul=float(n_per_part))
                    nc.scalar.mul(out=stats_loc[:, 4 + c:5 + c], in_=e2[:],
                                  mul=float(n_per_part))
                ps_red = psS.tile([1, 8], F32, tag="small", name="ps_red")
                nc.tensor.matmul(ps_red[:], ones_sb[:], stats_loc[:],
                                 start=True, stop=True)
                red_sb = stats.tile([1, 8], F32, tag="red")
                nc.vector.tensor_copy(red_sb[:], ps_red[:])

            cc_in = dram.tile([1, 8], F32)
            cc_out = dram.tile([1, 8], F32)
            nc.gpsimd.dma_start(out=cc_in[:], in_=red_sb[:])
            if no_collective:
                # timing-model variant: plain DRAM round trip instead of
                # the AllReduce (TimelineSim is single-core)
                nc.gpsimd.dma_start(out=cc_out[:], in_=cc_in[:])
            else:
                nc.gpsimd.collective_compute(
                    "AllReduce", mybir.AluOpType.add,
                    replica_groups=[list(range(n_cores))],
                    ins=[cc_in.opt()], outs=[cc_out.opt()])
            g_sb = stats.tile([1, 8], F32, tag="g")
            nc.gpsimd.dma_start(out=g_sb[:], in_=cc_out[:])

            # scale/shift: a = gamma/sqrt(var+eps), b = beta - mean*a
            mean = stats.tile([1, 4], F32, tag="mean")
            nc.scalar.mul(out=mean[:], in_=g_sb[0:1, 0:4], mul=1.0 / NTOT)
            e2g = stats.tile([1, 4], F32, tag="e2g")
            nc.scalar.mul(out=e2g[:], in_=g_sb[0:1, 4:8], mul=1.0 / NTOT)
            msqg = stats.tile([1, 4], F32, tag="msqg")
            nc.vector.tensor_mul(msqg[:], mean[:], mean[:])
            var = stats.tile([1, 4], F32, tag="var")
            nc.vector.tensor_sub(var[:], e2g[:], msqg[:])
            epst = stats.tile([1, 1], F32, tag="epst")
            nc.vector.memset(epst[:], EPS)
            sd = stats.tile([1, 4], F32, tag="sd")
            nc.scalar.activation(out=sd[:], in_=var[:],
                                 func=mybir.ActivationFunctionType.Sqrt,
                                 bias=epst[:], scale=1.0)
            rstd = stats.tile([1, 4], F32, tag="rstd")
            nc.vector.reciprocal(out=rstd[:], in_=sd[:])
            ab = stats.tile([1, 8], F32, tag="ab")
            nc.vector.tensor_mul(ab[0:1, 0:4], gb_sb[0:1, 0:4], rstd[:])
            tmp = stats.tile([1, 4], F32, tag="tmpb")
            nc.vector.tensor_mul(tmp[:], mean[:], ab[0:1, 0:4])
            nc.vector.tensor_sub(ab[0:1, 4:8], gb_sb[0:1, 4:8], tmp[:])

            ab_dram = dram.tile([1, 8], F32)
            nc.gpsimd.dma_start(out=ab_dram[:], in_=ab[:])
            ab_bc = stats.tile([128, 8], F32, tag="ab_bc")
            bc_ap = bass.AP(tensor=ab_dram.tensor, offset=ab_dram.offset,
                            ap=[[0, 128], [1, 8]])
            nc.gpsimd.dma_start(out=ab_bc[:], in_=bc_ap)

            # phase 2: back-transpose + fused BN affine + ReLU + DMA out
            grp = int(os.environ.get("KERNEL_GRP", "8"))
            with tc.tile_pool(name="spool", bufs=3) as spool, \
                 tc.tile_pool(name="psBT", bufs=int(os.environ.get("KERNEL_PSBT", "3")), space="PSUM") as psBT:
                for b in range(b_loc):
                    for c in range(CH):
                        st = spool.tile([128, F], F32, tag="stage")
                        for g in range(0, NS, grp):
                            ng = min(grp, NS - g)
                            pbt = psBT.tile([128, grp * 128], F32, tag="pbt")
                            for j in range(ng):
                                nc.tensor.transpose(
                                    pbt[:, j * 128:(j + 1) * 128],
                                    c_tiles[c][g + j][:, b * 128:(b + 1) * 128],
                                    ident_sb[:])
                            nc.scalar.activation(
                                out=st[:, g * 128:(g + ng) * 128],
                                in_=pbt[:, 0:ng * 128],
                                func=mybir.ActivationFunctionType.Relu,
                                scale=ab_bc[:, c:c + 1],
                                bias=ab_bc[:, 4 + c:5 + c])
                        ov = out_d.ap()[b, c]
                        # 3-way chunked store across the three DMA paths: a
                        # single ring moves ~30 GB/s; three chunks in flight
                        # on different rings run at ~125 GB/s aggregate.
                        nsp = int(os.environ.get("KERNEL_OSPLIT", "3"))
                        for sp_i in range(nsp):
                            p0 = sp_i * 127 // nsp
                            p1 = (sp_i + 1) * 127 // nsp
                            eng = dma_engs[(b * CH * nsp + c * nsp + sp_i) % 3]
                            eng.dma_start(
                                out=ov[0:127 * F].rearrange(
                                    "(a f) -> a f", f=F)[p0:p1, :],
                                in_=st[p0:p1, :])
                        nc.scalar.dma_start(
                            out=ov[127 * F:ML].rearrange("(a f) -> a f", a=1),
                            in_=st[127:128, 0:F - 2])

    return _finish(nc)


def _finish(nc):
    nc.compile()
    return nc


_CACHE = {}


def _get_nc(n_cores, b_loc, length, conv_f32r, no_collective=False, repeat=1):
    key = (n_cores, b_loc, length, conv_f32r, no_collective, repeat)
    if key not in _CACHE:
        _CACHE[key] = _build(*key)
    return _CACHE[key]


def _prepare_inputs(x, w1, b1, w2, b2, bn_gamma, bn_beta, kernels,
                    n_cores):
    x = np.ascontiguousarray(np.asarray(x, np.float32))
    bsz, _, length = x.shape
    s = _attention_weights(x[0, 0], np.asarray(w1, np.float32),
                           np.asarray(b1, np.float32),
                           np.asarray(w2, np.float32),
                           np.asarray(b2, np.float32))
    keff, ktail = _fir_taps(s, [np.asarray(k, np.float32) for k in kernels])
    A, Bm, D = _toeplitz_mats(keff, ktail)
    gb = np.concatenate([np.asarray(bn_gamma, np.float32),
                         np.asarray(bn_beta, np.float32)])[None, :]
    b_loc = bsz // n_cores
    in_maps = []
    for core in range(n_cores):
        in_maps.append({
            "x": x[core * b_loc:(core + 1) * b_loc, 0, :],
            "amat": A, "bmat": Bm, "dmat": D, "gb": gb,
        })
    return in_maps, b_loc, length


def _use_f32r():
    """f32r (TF32-like 1 cyc/row PE path) is the default: ~2e-4 max rel err
    against the 5e-3 tolerance, and 4x faster conv matmuls."""
    return os.environ.get("KERNEL_F32R", "1") == "1"


def run(inputs, n_cores=N_CORES, conv_f32r=None, trace=False):
    if conv_f32r is None:
        conv_f32r = _use_f32r()
    kernels = [inputs[f"k{i}"] for i in range(len(KS))]
    in_maps, b_loc, length = _prepare_inputs(
        inputs["x"], inputs["w1"], inputs["b1"], inputs["w2"], inputs["b2"],
        inputs["bn_gamma"], inputs["bn_beta"], kernels, n_cores)
    nc = _get_nc(n_cores, b_loc, length, conv_f32r)
    try:
        res = bass_utils.run_bass_kernel_spmd(
            nc, in_maps, core_ids=list(range(n_cores)), trace=trace)
    except ModuleNotFoundError:
        # no axon NTFF profiling hook in this container
        res = bass_utils.run_bass_kernel_spmd(
            nc, in_maps, core_ids=list(range(n_cores)), trace=False)
    out = np.concatenate([res.results[c]["out"] for c in range(n_cores)],
                         axis=0)
    return out, res


def kernel(**inputs):
    out, _ = run(inputs)
    return out



# revision 8
# speedup vs baseline: 1.3332x; 1.3332x over previous
"""Trainium2 Bass kernel for nn_DynamicConv (dense_cnn).

Math: the 12 scaled conv kernels (lengths 3..11, 1->4 channels) collapse by
linearity into one 11-tap FIR per channel; only the last 8 output positions
use masked (shorter) kernel sets, and the last 2 positions of the padded
length are dropped.  The attention MLP that produces the 12 softmax weights
reads only x[0] and is computed on host (0.03% of FLOPs); its result is baked
into the FIR taps passed to the device.

Device strategy (8 cores, batch-parallel, 4 batches/core):
  - x[b] viewed as 128 chunks x F (F = L/128).  PE transposes 128-column
    slices into "run-major" tiles XT[p, (b,a)] = x[b, a*F + c0 + p].
  - Conv = banded-Toeplitz matmul: psum[q, (b,a)] = A_c^T @ XT (+ B_c^T @
    XT_next rows 0..9 for taps crossing the 128 boundary), PSUM-accumulated.
  - bn_stats on every conv psum tile -> per-partition mean/M2; aggregated,
    reduced over partitions by a ones-matmul, AllReduce'd across the 8 cores
    (tiny [1,8] collective), then scale/shift = f(gamma, beta, mean, var).
  - Phase 2: PE back-transposes grouped 8-per-2-bank-PSUM so the fused
    BN-affine+ReLU runs as few wide ScalarE ops while copying PSUM->SBUF
    staging; contiguous 1MB output DMAs alternate across both HWDGE rings.
  - Default matmuls are exact fp32 (4 cyc/row on the PE); KERNEL_F32R=1
    switches the conv to the TF32 path (1 cyc/row, ~2e-4 max rel err).
"""

import os
import numpy as np

import concourse.bass as bass
import concourse.bacc as bacc
import concourse.tile as tile
from concourse import mybir
from concourse import bass_utils

KS = [3, 3, 3, 5, 5, 7, 7, 7, 9, 9, 11, 11]
B, L, CH = 32, 262144, 4
N_CORES = 8
EPS = 1e-5
MAXK = 11

F32 = mybir.dt.float32
F32R = mybir.dt.float32r


# ---------------------------------------------------------------- host math
def _attention_weights(x0, w1, b1, w2, b2):
    """softmax weights s[12] from batch element 0 (reference uses y[0,0])."""
    n = x0.shape[0]
    pooled = x0.reshape(256, n // 256).mean(axis=1)
    h = np.maximum(w1.astype(np.float32) @ pooled + b1, 0.0).astype(np.float32)
    z = (w2.astype(np.float32) @ h + b2).astype(np.float32)
    z = z - z.max()
    e = np.exp(z)
    return (e / e.sum()).astype(np.float32)


def _fir_taps(s, kernels):
    """K_eff[c, j] and the 8 masked tail variants K_tail[d][c, j]."""
    keff = np.zeros((CH, MAXK), np.float32)
    for i, k in enumerate(kernels):
        keff[:, : KS[i]] += s[i] * k[:, 0, :]
    ktail = np.zeros((8, CH, MAXK), np.float32)
    for d in range(8):
        for i, k in enumerate(kernels):
            if KS[i] <= 10 - d:
                ktail[d, :, : KS[i]] += s[i] * k[:, 0, :]
    return keff, ktail


def _toeplitz_mats(keff, ktail):
    """lhsT matrices for the banded conv, boundary corner, and masked tail."""
    A = np.zeros((CH, 128, 128), np.float32)
    Bm = np.zeros((CH, 10, 128), np.float32)
    for c in range(CH):
        for p in range(128):
            for q in range(max(0, p - (MAXK - 1)), p + 1):
                A[c, p, q] = keff[c, p - q]
        for r in range(10):
            for q in range(118 + r, 128):
                j = 128 - q + r
                if j < MAXK:
                    Bm[c, r, q] = keff[c, j]
    # correction for the last 128-column of the last slice: psum there holds
    # A.T@u (B side saw zeros).  D = T - A fixes q in [118,126) to the masked
    # tail value and exactly cancels q in {126,127} (dropped positions).
    D = np.zeros((CH, 128, 128), np.float32)
    for c in range(CH):
        T = np.zeros((128, 128), np.float32)
        for q in range(118, 126):
            d = q - 118
            for j in range(MAXK):
                p = q + j
                if p < 128:
                    T[p, q] = ktail[d, c, j]
        D[c, :, 118:] = T[:, 118:] - A[c, :, 118:]
    return A, Bm, D


# ---------------------------------------------------------------- device IR
def _build(n_cores, b_loc, length, conv_f32r, no_collective=False, repeat=1):
    """repeat>1 wraps the whole kernel body in a tc.For_i hardware loop —
    a timing-only variant that executes the identical (idempotent) kernel
    `repeat` times back-to-back on device, so steady-state per-pass time can
    be measured above the ~70-100 ms axon dispatch round-trip noise."""
    F = length // 128            # chunk length per partition row
    NS = F // 128                # number of 128-column slices
    ML = length - 2              # valid output length
    NTOT = float(n_cores * b_loc * ML)
    n_per_part = 512 * NS        # bn_stats element count per partition/channel

    nc = bacc.Bacc("TRN2", target_bir_lowering=False, debug=False,
                   num_devices=1 if no_collective else n_cores)

    x_d = nc.dram_tensor("x", [b_loc, length], F32, kind="ExternalInput")
    a_d = nc.dram_tensor("amat", [CH, 128, 128], F32, kind="ExternalInput")
    b_d = nc.dram_tensor("bmat", [CH, 10, 128], F32, kind="ExternalInput")
    d_d = nc.dram_tensor("dmat", [CH, 128, 128], F32, kind="ExternalInput")
    gb_d = nc.dram_tensor("gb", [1, 8], F32, kind="ExternalInput")
    out_d = nc.dram_tensor("out", [b_loc, CH, ML], F32, kind="ExternalOutput")

    wdt = F32R if conv_f32r else F32

    from contextlib import nullcontext
    with tile.TileContext(nc) as tc:
        with (tc.For_i(0, repeat, 1) if repeat > 1 else nullcontext()), \
             tc.tile_pool(name="singles", bufs=1) as singles, \
             tc.tile_pool(name="xpool", bufs=1) as xpool, \
             tc.tile_pool(name="cpool", bufs=NS * CH) as cpool, \
             tc.tile_pool(name="stats", bufs=1) as stats, \
             tc.tile_pool(name="dram", bufs=1, space="DRAM") as dram:

            ident = nc.inline_tensor(np.eye(128, dtype=np.float32), name="ident")
            ident_sb = singles.tile([128, 128], F32, tag="ident")
            nc.sync.dma_start(out=ident_sb, in_=ident.ap())
            ones = nc.inline_tensor(np.ones((128, 1), np.float32), name="ones")
            ones_sb = singles.tile([128, 1], F32, tag="ones")
            nc.sync.dma_start(out=ones_sb, in_=ones.ap())

            # conv weight matrices (cast to f32r on device when enabled)
            a_sb, b_sb = [], []
            for c in range(CH):
                at = singles.tile([128, 128], F32, tag=f"a{c}")
                nc.sync.dma_start(out=at, in_=a_d.ap()[c])
                bt = singles.tile([10, 128], F32, tag=f"b{c}")
                nc.sync.dma_start(out=bt, in_=b_d.ap()[c])
                if conv_f32r:
                    ar = singles.tile([128, 128], F32R, tag=f"ar{c}")
                    nc.vector.tensor_copy(ar[:], at[:])
                    br = singles.tile([10, 128], F32R, tag=f"br{c}")
                    nc.vector.tensor_copy(br[:], bt[:])
                    a_sb.append(ar)
                    b_sb.append(br)
                else:
                    a_sb.append(at)
                    b_sb.append(bt)
            d_sb = []
            for c in range(CH):
                dt_ = singles.tile([128, 128], F32, tag=f"d{c}", name=f"d{c}")
                nc.sync.dma_start(out=dt_, in_=d_d.ap()[c])
                if conv_f32r:
                    dr = singles.tile([128, 128], F32R, tag=f"dr{c}",
                                      name=f"dr{c}")
                    nc.vector.tensor_copy(dr[:], dt_[:])
                    d_sb.append(dr)
                else:
                    d_sb.append(dt_)
            gb_sb = singles.tile([1, 8], F32, tag="gb")
            nc.sync.dma_start(out=gb_sb, in_=gb_d.ap())

            # x tiles [128, F+10] with inter-chunk halo.  Loads round-robin
            # over the three DMA-issue paths (2 HWDGE rings + SWDGE): a single
            # ring sustains only ~30 GB/s here, three in parallel ~125 GB/s.
            dma_engs = [nc.sync, nc.scalar, nc.gpsimd]
            x_tiles = []
            for b in range(b_loc):
                xt = xpool.tile([128, F + 10], F32, tag=f"X{b}")
                xv = x_d.ap()[b].rearrange("(a f) -> a f", f=F)
                nq = int(os.environ.get("KERNEL_XSPLIT", "6"))
                for qi in range(nq):
                    f0, f1 = qi * F // nq, (qi + 1) * F // nq
                    eng = dma_engs[(b * nq + qi) % 3]
                    eng.dma_start(out=xt[:, f0:f1], in_=xv[:, f0:f1])
                nc.vector.memset(xt[:, F:F + 10], 0.0)
                nc.sync.dma_start(
                    out=xt[0:127, F:F + 10],
                    in_=x_d.ap()[b, F:length].rearrange(
                        "(a f) -> a f", f=F)[:, 0:10])
                x_tiles.append(xt)

            # per-channel bn_stats collection
            bnst = [stats.tile([128, NS, 6], F32, tag=f"bnst{c}", name=f"bnst{c}")
                    for c in range(CH)]
            c_tiles = [[None] * NS for _ in range(CH)]

            with tc.tile_pool(name="xt", bufs=int(os.environ.get("KERNEL_XT", "4"))) as xtp, \
                 tc.tile_pool(name="ht", bufs=1) as htp, \
                 tc.tile_pool(name="psT", bufs=2, space="PSUM") as psT, \
                 tc.tile_pool(name="psC", bufs=5, space="PSUM") as psC, \
                 tc.tile_pool(name="psS", bufs=1, space="PSUM") as psS:

                def make_xt(si):
                    c0 = si * 128
                    pst = psT.tile([128, 512], F32, tag="pst")
                    for b in range(b_loc):
                        nc.tensor.transpose(
                            pst[:, b * 128:(b + 1) * 128],
                            x_tiles[b][:, c0:c0 + 128], ident_sb[:])
                    xt4 = xtp.tile([128, 512], wdt, tag="xt4")
                    nc.scalar.copy(out=xt4[:], in_=pst[:])
                    return xt4

                def make_ht():
                    psh = psS.tile([16, 512], F32, tag="small", name="psh")
                    for b in range(b_loc):
                        nc.tensor.transpose(
                            psh[0:10, b * 128:(b + 1) * 128],
                            x_tiles[b][:, F:F + 10], ident_sb[:])
                    ht4 = htp.tile([10, 512], wdt, tag="ht4")
                    nc.vector.tensor_copy(ht4[:], psh[0:10, :])
                    return ht4

                xt_cur = make_xt(0)
                for si in range(NS):
                    last = si == NS - 1
                    xt_nxt = make_ht() if last else make_xt(si + 1)
                    for c in range(CH):
                        pc = psC.tile([128, 512], F32, tag="pc")
                        nc.tensor.matmul(pc[:], a_sb[c][:], xt_cur[:],
                                         start=True, stop=False)
                        nc.tensor.matmul(pc[:], b_sb[c][:], xt_nxt[0:10, :],
                                         start=False, stop=True)
                        if last:
                            pc3 = pc.rearrange("q (b a) -> q b a", b=b_loc)
                            u4 = xt_cur.rearrange(
                                "p (b a) -> p b a", b=b_loc)[:, :, 127]
                            ps_d = psS.tile([128, b_loc], F32, tag="small",
                                            name="ps_d")
                            nc.tensor.matmul(ps_d[:], d_sb[c][:], u4,
                                             start=True, stop=True)
                            sd_sb = stats.tile([128, b_loc], F32,
                                               tag="sdfix", bufs=2,
                                               name="sdfix")
                            nc.vector.tensor_copy(sd_sb[:], ps_d[:])
                            nc.vector.tensor_add(
                                pc3[:, :, 127], pc3[:, :, 127], sd_sb[:])
                        nc.vector.bn_stats(out=bnst[c][:, si, :], in_=pc[:])
                        ct = cpool.tile([128, 512], F32, tag="ct")
                        n_dve = int(os.environ.get("KERNEL_CDVE", "1"))
                        if c >= n_dve:
                            nc.scalar.copy(out=ct[:], in_=pc[:])
                        else:
                            nc.vector.tensor_copy(ct[:], pc[:])
                        c_tiles[c][si] = ct
                    xt_cur = xt_nxt

                # ---- stats finalize + collective
                stats_loc = stats.tile([128, 8], F32, tag="stats_loc")
                for c in range(CH):
                    mv = stats.tile([128, 2], F32, tag=f"mv{c}")
                    nc.vector.bn_aggr(out=mv[:], in_=bnst[c][:])
                    msq = stats.tile([128, 1], F32, tag=f"msq{c}")
                    nc.vector.tensor_mul(msq[:], mv[:, 0:1], mv[:, 0:1])
                    e2 = stats.tile([128, 1], F32, tag=f"e2{c}")
                    nc.vector.tensor_add(e2[:], mv[:, 1:2], msq[:])
                    nc.scalar.mul(out=stats_loc[:, c:c + 1], in_=mv[:, 0:1],
                                  mul=float(n_per_part))
                    nc.scalar.mul(out=stats_loc[:, 4 + c:5 + c], in_=e2[:],
                                  mul=float(n_per_part))
                ps_red = psS.tile([1, 8], F32, tag="small", name="ps_red")
                nc.tensor.matmul(ps_red[:], ones_sb[:], stats_loc[:],
                                 start=True, stop=True)
                red_sb = stats.tile([1, 8], F32, tag="red")
                nc.vector.tensor_copy(red_sb[:], ps_red[:])

            cc_in = dram.tile([1, 8], F32)
            cc_out = dram.tile([1, 8], F32)
            nc.gpsimd.dma_start(out=cc_in[:], in_=red_sb[:])
            if no_collective:
                # timing-model variant: plain DRAM round trip instead of
                # the AllReduce (TimelineSim is single-core)
                nc.gpsimd.dma_start(out=cc_out[:], in_=cc_in[:])
            else:
                nc.gpsimd.collective_compute(
                    "AllReduce", mybir.AluOpType.add,
                    replica_groups=[list(range(n_cores))],
                    ins=[cc_in.opt()], outs=[cc_out.opt()])
            g_sb = stats.tile([1, 8], F32, tag="g")
            nc.gpsimd.dma_start(out=g_sb[:], in_=cc_out[:])

            # scale/shift: a = gamma/sqrt(var+eps), b = beta - mean*a
            mean = stats.tile([1, 4], F32, tag="mean")
            nc.scalar.mul(out=mean[:], in_=g_sb[0:1, 0:4], mul=1.0 / NTOT)
            e2g = stats.tile([1, 4], F32, tag="e2g")
            nc.scalar.mul(out=e2g[:], in_=g_sb[0:1, 4:8], mul=1.0 / NTOT)
            msqg = stats.tile([1, 4], F32, tag="msqg")
            nc.vector.tensor_mul(msqg[:], mean[:], mean[:])
            var = stats.tile([1, 4], F32, tag="var")
            nc.vector.tensor_sub(var[:], e2g[:], msqg[:])
            epst = stats.tile([1, 1], F32, tag="epst")
            nc.vector.memset(epst[:], EPS)
            sd = stats.tile([1, 4], F32, tag="sd")
            nc.scalar.activation(out=sd[:], in_=var[:],
                                 func=mybir.ActivationFunctionType.Sqrt,
                                 bias=epst[:], scale=1.0)
            rstd = stats.tile([1, 4], F32, tag="rstd")
            nc.vector.reciprocal(out=rstd[:], in_=sd[:])
            ab = stats.tile([1, 8], F32, tag="ab")
            nc.vector.tensor_mul(ab[0:1, 0:4], gb_sb[0:1, 0:4], rstd[:])
            tmp = stats.tile([1, 4], F32, tag="tmpb")
            nc.vector.tensor_mul(tmp[:], mean[:], ab[0:1, 0:4])
            nc.vector.tensor_sub(ab[0:1, 4:8], gb_sb[0:1, 4:8], tmp[:])

            ab_dram = dram.tile([1, 8], F32)
            nc.gpsimd.dma_start(out=ab_dram[:], in_=ab[:])
            ab_bc = stats.tile([128, 8], F32, tag="ab_bc")
            bc_ap = bass.AP(tensor=ab_dram.tensor, offset=ab_dram.offset,
                            ap=[[0, 128], [1, 8]])
            nc.gpsimd.dma_start(out=ab_bc[:], in_=bc_ap)

            # phase 2: back-transpose + fused BN affine + ReLU + DMA out
            grp = int(os.environ.get("KERNEL_GRP", "8"))
            with tc.tile_pool(name="spool", bufs=3) as spool, \
                 tc.tile_pool(name="psBT", bufs=int(os.environ.get("KERNEL_PSBT", "3")), space="PSUM") as psBT:
                for b in range(b_loc):
                    for c in range(CH):
                        st = spool.tile([128, F], F32, tag="stage")
                        for g in range(0, NS, grp):
                            ng = min(grp, NS - g)
                            pbt = psBT.tile([128, grp * 128], F32, tag="pbt")
                            for j in range(ng):
                                nc.tensor.transpose(
                                    pbt[:, j * 128:(j + 1) * 128],
                                    c_tiles[c][g + j][:, b * 128:(b + 1) * 128],
                                    ident_sb[:])
                            nc.scalar.activation(
                                out=st[:, g * 128:(g + ng) * 128],
                                in_=pbt[:, 0:ng * 128],
                                func=mybir.ActivationFunctionType.Relu,
                                scale=ab_bc[:, c:c + 1],
                                bias=ab_bc[:, 4 + c:5 + c])
                        ov = out_d.ap()[b, c]
                        # 3-way chunked store across the three DMA paths: a
                        # single ring moves ~30 GB/s; three chunks in flight
                        # on different rings run at ~125 GB/s aggregate.
                        nsp = int(os.environ.get("KERNEL_OSPLIT", "6"))
                        for sp_i in range(nsp):
                            p0 = sp_i * 127 // nsp
                            p1 = (sp_i + 1) * 127 // nsp
                            eng = dma_engs[(b * CH * nsp + c * nsp + sp_i) % 3]
                            eng.dma_start(
                                out=ov[0:127 * F].rearrange(
                                    "(a f) -> a f", f=F)[p0:p1, :],
                                in_=st[p0:p1, :])
                        nc.scalar.dma_start(
                            out=ov[127 * F:ML].rearrange("(a f) -> a f", a=1),
                            in_=st[127:128, 0:F - 2])

    return _finish(nc)


def _finish(nc):
    nc.compile()
    return nc


_CACHE = {}


def _get_nc(n_cores, b_loc, length, conv_f32r, no_collective=False, repeat=1):
    key = (n_cores, b_loc, length, conv_f32r, no_collective, repeat)
    if key not in _CACHE:
        _CACHE[key] = _build(*key)
    return _CACHE[key]


def _prepare_inputs(x, w1, b1, w2, b2, bn_gamma, bn_beta, kernels,
                    n_cores):
    x = np.ascontiguousarray(np.asarray(x, np.float32))
    bsz, _, length = x.shape
    s = _attention_weights(x[0, 0], np.asarray(w1, np.float32),
                           np.asarray(b1, np.float32),
                           np.asarray(w2, np.float32),
                           np.asarray(b2, np.float32))
    keff, ktail = _fir_taps(s, [np.asarray(k, np.float32) for k in kernels])
    A, Bm, D = _toeplitz_mats(keff, ktail)
    gb = np.concatenate([np.asarray(bn_gamma, np.float32),
                         np.asarray(bn_beta, np.float32)])[None, :]
    b_loc = bsz // n_cores
    in_maps = []
    for core in range(n_cores):
        in_maps.append({
            "x": x[core * b_loc:(core + 1) * b_loc, 0, :],
            "amat": A, "bmat": Bm, "dmat": D, "gb": gb,
        })
    return in_maps, b_loc, length


def _use_f32r():
    """f32r (TF32-like 1 cyc/row PE path) is the default: ~2e-4 max rel err
    against the 5e-3 tolerance, and 4x faster conv matmuls."""
    return os.environ.get("KERNEL_F32R", "1") == "1"


def run(inputs, n_cores=N_CORES, conv_f32r=None, trace=False):
    if conv_f32r is None:
        conv_f32r = _use_f32r()
    kernels = [inputs[f"k{i}"] for i in range(len(KS))]
    in_maps, b_loc, length = _prepare_inputs(
        inputs["x"], inputs["w1"], inputs["b1"], inputs["w2"], inputs["b2"],
        inputs["bn_gamma"], inputs["bn_beta"], kernels, n_cores)
    nc = _get_nc(n_cores, b_loc, length, conv_f32r)
    try:
        res = bass_utils.run_bass_kernel_spmd(
            nc, in_maps, core_ids=list(range(n_cores)), trace=trace)
    except ModuleNotFoundError:
        # no axon NTFF profiling hook in this container
        res = bass_utils.run_bass_kernel_spmd(
            nc, in_maps, core_ids=list(range(n_cores)), trace=False)
    out = np.concatenate([res.results[c]["out"] for c in range(n_cores)],
                         axis=0)
    return out, res


def kernel(**inputs):
    out, _ = run(inputs)
    return out

